# revision 7
# baseline (speedup 1.0000x reference)
"""Group VQ (vq_codebook) Trainium2 Bass kernel, v3.

Data-parallel over batch B=16 across 8 cores (2 batches/core). Per core,
16 (batch, group) slabs x 32 token-tiles of 125 tokens. Scores
s[t,k] = 2 x.e_k - |e_k|^2 computed as four fp16 cross-term matmuls
(x and Etilde split hi/lo fp16; all four hi/lo products in fp32 PSUM give
~2^-24 accuracy; the PE stays below the DVE bottleneck so this is free). The argmax over the 1024 codes is computed fully on
device with two DVE ops: tensor_scalar(ps*1, accum op1=max) gives an
SBUF score copy plus the max m in one PSUM pass, then
scalar_tensor_tensor (s is_ge m) mult iota, whose SUM-accumulator is
exactly the argmax index (the unique fp32 max contributes k*1, all
others 0; bit-equal score ties sum indices -- ~1 token per 512K,
clipped host-side). Host just gathers code vectors by index.

DMA layout is descriptor-efficient: x is repacked host-side to
[65, chunk, hi|lo slabs, T] fp16 so each of 4 chunk-DMAs moves
64KB-contiguous runs per partition (260 descriptors total vs ~17K for
naive [65, 500]-tile loads); codebooks arrive in one [65, 16384] DMA
(65 desc); output is a single [125, 512] fp32 index tensor (125 desc).
"""
import sys
import numpy as np
from contextlib import ExitStack

sys.path.insert(0, "/opt/trn_rl_repo")

B, C, F, T = 16, 2, 256, 4000
G, K, D = 8, 1024, 64
NCORES = 8
NB = B // NCORES            # batches per core = 2
NSLAB = NB * G              # 16 (b_loc, g) slabs per core
TT = 125                    # tokens per tile
NT = T // TT                # 32 tiles per slab
SLAB_PER_CHUNK = 4
NCHUNK = NSLAB // SLAB_PER_CHUNK   # 4
CHUNK_W = SLAB_PER_CHUNK * T * 2   # 32000 fp16 elems per partition (hi|lo)

_compiled = None


def _build_program():
    import concourse.bass as bass
    import concourse.tile as tile
    from concourse import bacc, mybir

    nc = bacc.Bacc(
        "TRN2",
        target_bir_lowering=False,
        debug=False,
        enable_asserts=False,
        num_devices=NCORES,
    )
    f32 = mybir.dt.float32
    f16 = mybir.dt.float16
    i32 = mybir.dt.int32
    xa = nc.dram_tensor("xa", [65, NCHUNK * CHUNK_W], f16,
                        kind="ExternalInput").ap()
    et = nc.dram_tensor("et", [65, 2 * G * K], f16, kind="ExternalInput").ap()
    om = nc.dram_tensor("om", [TT, NSLAB * NT], f32, kind="ExternalOutput").ap()

    with tile.TileContext(nc) as tc, ExitStack() as ctx:
        epool = ctx.enter_context(tc.tile_pool(name="e", bufs=1))
        xpool = ctx.enter_context(tc.tile_pool(name="x", bufs=2))
        cpool = ctx.enter_context(tc.tile_pool(name="c", bufs=1))
        ppool = ctx.enter_context(
            tc.tile_pool(name="ps", bufs=4, space=bass.MemorySpace.PSUM))
        spool = ctx.enter_context(tc.tile_pool(name="ssb", bufs=2))
        scrpool = ctx.enter_context(tc.tile_pool(name="scr", bufs=2))
        mpool = ctx.enter_context(tc.tile_pool(name="m", bufs=2))

        e_sb = epool.tile([65, 2 * G * K], f16)
        nc.sync.dma_start(e_sb[:], et)

        iota_i = cpool.tile([TT, K], i32)
        nc.gpsimd.iota(iota_i[:], pattern=[[1, K]], base=0,
                       channel_multiplier=0)
        iota_f = cpool.tile([TT, K], f32)
        nc.vector.tensor_scalar(iota_f[:], iota_i[:], 1.0, None,
                                op0=mybir.AluOpType.mult)
        om_sb = cpool.tile([TT, NSLAB * NT], f32)

        for c in range(NCHUNK):
            x_sb = xpool.tile([65, CHUNK_W], f16, tag="x")
            nc.sync.dma_start(x_sb[:], xa[:, c * CHUNK_W:(c + 1) * CHUNK_W])
            for s in range(SLAB_PER_CHUNK):
                bg = c * SLAB_PER_CHUNK + s
                g = bg % G
                eh = e_sb[:, g * K:(g + 1) * K]
                el = e_sb[:, (G + g) * K:(G + g + 1) * K]
                xh_slab = x_sb[:, s * T:(s + 1) * T]
                xl_slab = x_sb[:, (SLAB_PER_CHUNK + s) * T:
                               (SLAB_PER_CHUNK + s + 1) * T]
                for t in range(NT):
                    xh_t = xh_slab[:, t * TT:(t + 1) * TT]
                    xl_t = xl_slab[:, t * TT:(t + 1) * TT]
                    ps = ppool.tile([TT, K], f32)
                    lo, hi = slice(0, 512), slice(512, K)
                    nc.tensor.matmul(ps[:, lo], xh_t, eh[:, lo],
                                     start=True, stop=False)
                    nc.tensor.matmul(ps[:, hi], xh_t, eh[:, hi],
                                     start=True, stop=False)
                    nc.tensor.matmul(ps[:, lo], xh_t, el[:, lo],
                                     start=False, stop=False)
                    nc.tensor.matmul(ps[:, hi], xh_t, el[:, hi],
                                     start=False, stop=False)
                    nc.tensor.matmul(ps[:, lo], xl_t, eh[:, lo],
                                     start=False, stop=False)
                    nc.tensor.matmul(ps[:, hi], xl_t, eh[:, hi],
                                     start=False, stop=False)
                    nc.tensor.matmul(ps[:, lo], xl_t, el[:, lo],
                                     start=False, stop=True)
                    nc.tensor.matmul(ps[:, hi], xl_t, el[:, hi],
                                     start=False, stop=True)

                    # one PSUM pass: copy scores to SBUF AND take the
                    # max (tensor_scalar accum reduces with op1)
                    m = mpool.tile([TT, 1], f32)
                    s_sb = spool.tile([TT, K], f32)
                    nc.vector.tensor_scalar(s_sb[:], ps[:], 1.0, None,
                                            op0=mybir.AluOpType.mult,
                                            op1=mybir.AluOpType.max,
                                            accum_out=m[:])
                    scr = scrpool.tile([TT, K], f16)
                    col = bg * NT + t
                    # accum = sum((s >= m) * k) = argmax index (exact for
                    # a unique fp32 max; bit-equal ties sum indices, ~1
                    # token per 512K, clipped host-side); reads SBUF at 2x
                    nc.vector.scalar_tensor_tensor(
                        scr[:], s_sb[:], m[:], iota_f[:],
                        op0=mybir.AluOpType.is_ge, op1=mybir.AluOpType.mult,
                        accum_out=om_sb[:, col:col + 1])
        nc.sync.dma_start(om, om_sb[:])

    nc.compile()
    return nc


def _get_compiled():
    global _compiled
    if _compiled is None:
        _compiled = _build_program()
    return _compiled


def _prep_inputs(x, codebooks):
    # x: [B,C,F,T] fp32 -> per-core xa [65, NCHUNK*CHUNK_W] fp16
    # layout: [d(65), chunk, hi|lo, slab-in-chunk, t] with slab bg = b_loc*8+g
    xc = np.ascontiguousarray(
        x.reshape(NCORES, NB, G, D, T).transpose(0, 3, 1, 2, 4)
    ).reshape(NCORES, D, NCHUNK, SLAB_PER_CHUNK * T)     # [core, 64, bg-chunks, t]
    hi = xc.astype(np.float16)
    lo = (xc - hi.astype(np.float32)).astype(np.float16)
    xa = np.empty((NCORES, 65, NCHUNK, 2, SLAB_PER_CHUNK * T), np.float16)
    xa[:, :D, :, 0] = hi
    xa[:, :D, :, 1] = lo
    xa[:, D, :, 0] = np.float16(1.0)
    xa[:, D, :, 1] = np.float16(0.0)
    xa = xa.reshape(NCORES, 65, NCHUNK * CHUNK_W)

    # Etilde: [65, 2*G*K] fp16: rows 0..63 = 2*E^T, row 64 = -|e|^2,
    # cols = [hi: g*K..] then [lo: (G+g)*K..]
    etf = np.empty((G, 65, K), dtype=np.float32)
    etf[:, :D, :] = 2.0 * np.transpose(codebooks, (0, 2, 1))
    etf[:, D, :] = -(codebooks.astype(np.float32) ** 2).sum(-1)
    eh = etf.astype(np.float16)
    el = (etf - eh.astype(np.float32)).astype(np.float16)
    et = np.empty((65, 2 * G * K), np.float16)
    for g in range(G):
        et[:, g * K:(g + 1) * K] = eh[g]
        et[:, (G + g) * K:(G + g + 1) * K] = el[g]
    return xa, et


def run_device(x, codebooks, trace=False):
    from concourse.bass_utils import run_bass_kernel_spmd

    nc = _get_compiled()
    xa, et = _prep_inputs(np.asarray(x, np.float32),
                          np.asarray(codebooks, np.float32))
    in_maps = [{"xa": np.ascontiguousarray(xa[core]), "et": et}
               for core in range(NCORES)]
    return run_bass_kernel_spmd(nc, in_maps, list(range(NCORES)), trace=trace)


def kernel(x, codebooks):
    x = np.asarray(x, dtype=np.float32)
    codebooks = np.asarray(codebooks, dtype=np.float32)
    res = run_device(x, codebooks)
    # om [125, NSLAB*NT]: om[p, bg*NT + t] = 1023 - idx of token t*125+p
    idx = np.empty((B, G, T), dtype=np.int64)
    for core in range(NCORES):
        o = res.results[core]["om"]                       # [125, 512]
        kk = np.rint(o).astype(np.int64)
        kk = kk.reshape(TT, NSLAB, NT).transpose(1, 2, 0)  # [bg, t, p]
        kk = kk.reshape(NB, G, T)                          # token = t*125+p
        idx[core * NB:(core + 1) * NB] = kk
    np.clip(idx, 0, K - 1, out=idx)
    # gather code vectors: q[b,g,:,t] = codebooks[g, idx[b,g,t]]
    q = np.empty((B, G, D, T), dtype=np.float32)
    for g in range(G):
        q[:, g] = codebooks[g][idx[:, g]].transpose(0, 2, 1)
    q = q.reshape(B, C, F, T)
    x_q = x + (q - x)
    return x_q, q
